# revision 62
# baseline (speedup 1.0000x reference)
"""Trainium2 Bass kernel: masked-LSTM readout over to_dense_batch'd graphs.

v5 strategy (8 NeuronCores, SPMD single program):
 - Host: per-graph lengths from sorted `index`; graphs globally sorted by
   length (desc) and dealt round-robin to 8 cores, so all cores share one
   step schedule N_t. Host densifies x into a block-major padded tensor per
   core (fp16), feature-major [64, rows].
 - Device per step: rhs = [h_{t-1} ; x_t] stacked on 128 partitions (h
   written into the x tile's top half by the previous step), so each
   gate-pair needs ONE matmul with contract 128. All four gates use one
   sigmoid form (g via s=sig(2g), per-partition ACT scale (1;2)); cell
   update in fp16: TT ops in DVE 2x mode, tanh(c) on ACT, f*c on GpSimd.
 - Two independent column pieces run PHASE-OFFSET by one step (piece0 a
   full step ahead in every engine queue) so the serial LSTM chain of one
   piece hides behind the other piece's engine work.
 - Final h snapshot via predicated copy at each graph's last valid step.
"""

import numpy as np

MAXLEN = 100
B = 8192
NCORES = 8
G = B // NCORES          # graph columns per core
H = 64
F = 64
TW = 24                  # steps per time block
CHUNK = 512              # psum bank width (f32 cols)

_CACHE = {}


def _split(rng, cuts):
    """Split [lo,hi) at the given cut points into ordered segments."""
    lo, hi = rng
    pts = sorted({lo, hi, *[c for c in cuts if lo < c < hi]})
    return list(zip(pts[:-1], pts[1:]))


def _build_and_compile(schedule, weights):
    import concourse.bacc as bacc
    import concourse.mybir as mybir
    from concourse import tile

    N_t, blocks, snap, MW = schedule
    fp16 = mybir.dt.float16
    f32 = mybir.dt.float32
    T_end = len(N_t)
    ROWS_TOT = sum(Wb * nst for (_, nst, Wb, _) in blocks)
    XT_W = max(Wb * nst for (_, nst, Wb, _) in blocks)

    nc = bacc.Bacc("TRN2", target_bir_lowering=False)
    xd_d = nc.dram_tensor("xd", [64, ROWS_TOT], fp16, kind="ExternalInput")
    msk_d = nc.dram_tensor("msk", [64, max(MW, 1)], mybir.dt.uint8, kind="ExternalInput")
    out_d = nc.dram_tensor("outh", [64, G], fp16, kind="ExternalOutput")
    # gate pairing: bank A = (f, g) [g-weights pre-doubled], bank B = (o, i)
    wa_d = nc.dram_tensor("wa", [128, 128], fp16, kind="ExternalInput")
    wb_d = nc.dram_tensor("wb", [128, 128], fp16, kind="ExternalInput")
    ba_d = nc.dram_tensor("ba", [128, 1], f32, kind="ExternalInput")
    bb_d = nc.dram_tensor("bb", [128, 1], f32, kind="ExternalInput")

    Sig = mybir.ActivationFunctionType.Sigmoid
    Tanh = mybir.ActivationFunctionType.Tanh
    Mult = mybir.AluOpType.mult
    Add = mybir.AluOpType.add

    blk_of = {}
    for bi, (t0, nst, Wb, row0) in enumerate(blocks):
        for ts in range(nst):
            blk_of[t0 + ts] = (bi, ts)

    # per-step metadata
    meta = []
    for t in range(T_end):
        n = N_t[t]
        if n == 0:
            break
        bi, ts = blk_of[t]
        Wb = blocks[bi][2]
        if n > 32:
            # balanced split at every width: both chains get ~n/2 columns so
            # they hide each other even in the mid-taper (n slightly > 512)
            m = min(CHUNK, (n // 2 + 15) & ~15)
            pieces = [(0, m), (m, n)]
        else:
            pieces = [(0, n)]
        if t + 1 < T_end and N_t[t + 1] > 0:
            nbi, nts = blk_of[t + 1]
            base_n = nts * blocks[nbi][2]
            wA = min(n, blocks[nbi][2])
        else:
            nbi, base_n, wA = None, 0, 0
        meta.append(dict(t=t, n=n, bi=bi, base=ts * Wb, pieces=pieces,
                         nbi=nbi, base_n=base_n, wA=wA))
    T = len(meta)

    with tile.TileContext(nc) as tc:
        with tc.tile_pool(name="state", bufs=1) as sp, \
             tc.tile_pool(name="xblk", bufs=2) as xp, \
             tc.tile_pool(name="psum", bufs=2, space="PSUM") as pp, \
             tc.tile_pool(name="gates", bufs=2) as gp:
            wa = sp.tile([128, 128], fp16)
            nc.sync.dma_start(out=wa, in_=wa_d.ap())
            wb = sp.tile([128, 128], fp16)
            nc.sync.dma_start(out=wb, in_=wb_d.ap())
            ba = sp.tile([128, 1], f32)
            nc.sync.dma_start(out=ba, in_=ba_d.ap())
            bb = sp.tile([128, 1], f32)
            nc.sync.dma_start(out=bb, in_=bb_d.ap())
            mskt = sp.tile([64, max(MW, 1)], mybir.dt.uint8)
            nc.sync.dma_start(out=mskt, in_=msk_d.ap())

            c = sp.tile([64, 1024], fp16, name="c")
            t1 = sp.tile([64, 1024], fp16, name="t1")
            t2 = sp.tile([64, 1024], fp16, name="t2")
            gt = sp.tile([128, 1024], fp16, name="gt")   # g~ on partitions 64:128
            tcc = sp.tile([64, 1024], fp16, name="tcc")
            hs = sp.tile([64, 1024], fp16, name="hs")
            outh = sp.tile([64, 1024], fp16, name="outh")
            nc.vector.memset(c[:, :], 0.0)
            nc.vector.memset(outh[:, :], 0.0)
            nc.vector.memset(hs[:, :], 0.0)

            xts = {}
            sgs = {}   # live sigma-output tiles keyed (t, pi)
            pss = {}
            pending_cps = []

            def make_xt(bi2):
                if bi2 in xts or bi2 >= len(blocks):
                    return
                _, nst2, Wb2, row02 = blocks[bi2]
                rows2 = Wb2 * nst2
                xt2 = xp.tile([128, XT_W], fp16, tag="xt", name=f"xt{bi2}")
                xts[bi2] = xt2
                # x occupies partitions 64:128; h occupies 0:64
                nc.sync.dma_start(out=xt2[64:128, 0:rows2],
                                  in_=xd_d.ap()[:, row02:row02 + rows2])
                if bi2 == 0:
                    nc.vector.memset(xt2[0:64, 0:Wb2], 0.0)

            def emit_A(k, pi):
                """MM + sigma + front of DVE chain for (step k, piece pi)."""
                st = meta[k]
                if pi >= len(st["pieces"]):
                    return
                lo, hi = st["pieces"][pi]
                w = hi - lo
                bi, base = st["bi"], st["base"]
                if pi == 0:
                    make_xt(bi)
                    make_xt(bi + 1)
                xt = xts[bi]
                # separate psum tiles per gate bank so sigA only waits on MM_A
                psa = pp.tile([128, 512], f32, tag=f"pa{pi}", name=f"pa{pi}_{k}")
                psb = pp.tile([128, 512], f32, tag=f"pb{pi}", name=f"pb{pi}_{k}")
                pss[(k, pi)] = (psa, psb)
                # bank A = (f, g): unlocks TS->TT2 and TT1 immediately
                nc.tensor.matmul(out=psa[:, 0:w], lhsT=wa[:, :],
                                 rhs=xt[:, base + lo:base + hi],
                                 start=True, stop=True)
                nc.tensor.matmul(out=psb[:, 0:w], lhsT=wb[:, :],
                                 rhs=xt[:, base + lo:base + hi],
                                 start=True, stop=True)
                sg = gp.tile([128, 1024], fp16, tag=f"sg{pi}", name=f"sg{pi}_{k}")
                sgs[(k, pi)] = sg
                nc.scalar.activation(out=sg[:, 0:w], in_=psa[:, 0:w],
                                     func=Sig, bias=ba[:, :])
                nc.scalar.activation(out=sg[:, CHUNK:CHUNK + w],
                                     in_=psb[:, 0:w],
                                     func=Sig, bias=bb[:, :])
                # g~ = 2*s - 1 (s = sig(2g) at A rows 64:128); on GpSimd: it
                # hides under sigB's ACT time, and Pool is otherwise idle
                nc.gpsimd.tensor_scalar(
                    out=gt[64:128, lo:hi], in0=sg[64:128, 0:w],
                    scalar1=2.0, scalar2=-1.0, op0=Mult, op1=Add)
                # t1 = sig(f) * c
                nc.vector.tensor_tensor(
                    out=t1[:, lo:hi], in0=sg[0:64, 0:w],
                    in1=c[:, lo:hi], op=Mult)
                # t2 = sig(i) * g~   (i = B rows 64:128)
                nc.vector.tensor_tensor(
                    out=t2[:, lo:hi], in0=sg[64:128, CHUNK:CHUNK + w],
                    in1=gt[64:128, lo:hi], op=Mult)
                nc.vector.tensor_tensor(
                    out=c[:, lo:hi], in0=t1[:, lo:hi],
                    in1=t2[:, lo:hi], op=Add)

            def emit_B(k, pi):
                """tanh + h-update + snapshot for (step k, piece pi)."""
                st = meta[k]
                if pi >= len(st["pieces"]):
                    return
                lo, hi = st["pieces"][pi]
                wA, base_n, nbi = st["wA"], st["base_n"], st["nbi"]
                xt_n = xts[nbi] if nbi is not None else None
                sg = sgs[(k, pi)]
                nc.scalar.activation(out=tcc[:, lo:hi], in_=c[:, lo:hi],
                                     func=Tanh)
                # h = sig(o) * tanh(c)   (o = B rows 0:64)
                for (a, b_) in _split((lo, hi), [wA]):
                    if b_ <= wA:
                        dst = xt_n[0:64, base_n + a:base_n + b_]
                    else:
                        dst = hs[:, a:b_]
                    nc.vector.tensor_tensor(
                        out=dst, in0=sg[0:64, CHUNK + a - lo:CHUNK + b_ - lo],
                        in1=tcc[:, a:b_], op=Mult)
                # snapshot graphs whose sequence ends at step k: batched and
                # flushed at block end to keep CPs out of the hot DVE window
                for (slo, shi, moff) in snap[st["t"]]:
                    ilo, ihi = max(slo, lo), min(shi, hi)
                    if ihi <= ilo:
                        continue
                    for (a, b_) in _split((ilo, ihi), [wA]):
                        if b_ <= wA:
                            src = xt_n[0:64, base_n + a:base_n + b_]
                        else:
                            src = hs[:, a:b_]
                        pending_cps.append((a, b_, moff + a - slo, src))

            def flush_cps():
                for (a, b_, mo, src) in pending_cps:
                    nc.vector.copy_predicated(
                        out=outh[:, a:b_], mask=mskt[:, mo:mo + b_ - a],
                        data=src)
                pending_cps.clear()

            # phase-offset emission: piece0 runs one step ahead of piece1.
            # Each piece's B-part (tanh/h-update) queues directly behind its
            # own A-part chain so TT4 is never stuck behind the other piece.
            emit_A(0, 0)
            for k in range(T):
                emit_B(k, 0)
                emit_A(k, 1)
                emit_B(k, 1)
                if k + 1 < T:
                    emit_A(k + 1, 0)
                if k + 1 >= T or meta[k + 1]["bi"] != meta[k]["bi"]:
                    flush_cps()

            nc.sync.dma_start(out=out_d.ap()[:, 0:G], in_=outh[:, 0:G])
    nc.compile()
    return nc


def _plan(lens):
    """Global schedule from capped lengths [B]."""
    order = np.argsort(-lens, kind="stable")
    lens_sorted = lens[order]
    T_end = int(lens_sorted.max())
    len_c = lens_sorted.reshape(G, NCORES).T            # [NCORES, G]
    t_ax = np.arange(T_end + 1)
    n_c = (len_c[:, :, None] > t_ax[None, None, :]).sum(axis=1)
    N_t = n_c.max(axis=0)                               # [T_end+1], N_t[T_end]==0
    blocks = []
    row0 = 0
    t0 = 0
    while t0 < T_end:
        # small first block so compute starts as soon as its DMA lands
        nsteps = min(4 if t0 == 0 else TW, T_end - t0)
        Wb = max(16, int(np.ceil(N_t[t0] / 16) * 16))
        blocks.append((t0, nsteps, Wb, row0))
        row0 += Wb * nsteps
        t0 += nsteps
    snap = []
    moff = 0
    mask_cols = []
    for t in range(T_end):
        nt1 = n_c[:, t + 1]
        lo = int(nt1.min())
        hi = int(n_c[:, t].max())
        pieces = []
        if hi > lo:
            m = np.zeros((NCORES, hi - lo), np.uint8)
            for cc in range(NCORES):
                a, b_ = int(nt1[cc]), int(n_c[cc, t])
                m[cc, max(a - lo, 0):max(b_ - lo, 0)] = 1
            mask_cols.append(m)
            pieces.append((lo, hi, moff))
            moff += hi - lo
        snap.append(pieces)
    masks = (np.concatenate(mask_cols, axis=1) if mask_cols
             else np.zeros((NCORES, 1), np.uint8))
    return order, len_c, n_c, [int(x) for x in N_t[:T_end]], blocks, snap, masks


LAST_RUN = {}


def _install_ntff_shim():
    import sys, types
    if "antenv.axon_hooks" in sys.modules:
        return
    try:
        from trn_agent_boot.trn_boot import _ntff_profile_via_ctypes
        hook = _ntff_profile_via_ctypes("/opt/axon/libaxon_pjrt.so")
    except Exception:
        hook = None
    m = types.ModuleType("antenv.axon_hooks")
    m._hook = hook
    m.get_axon_ntff_profile_hook = lambda: m._hook
    m.set_axon_ntff_profile_hook = lambda h: setattr(m, "_hook", h)
    sys.modules["antenv.axon_hooks"] = m


def kernel(x, W_ih, W_hh, b_ih, b_hh, index, dim_size, _trace=False):
    from concourse.bass_utils import run_bass_kernel_spmd
    if _trace:
        import concourse.bass_utils as _bu
        _install_ntff_shim()
        _bu.upload_artifacts = lambda d: d

    x = np.asarray(x)
    index = np.asarray(index).astype(np.int64)
    W_ih = np.asarray(W_ih, dtype=np.float32)
    W_hh = np.asarray(W_hh, dtype=np.float32)
    b_ih = np.asarray(b_ih, dtype=np.float32)
    b_hh = np.asarray(b_hh, dtype=np.float32)

    assert int(dim_size) == B, f"kernel hardcodes B={B}, got dim_size={int(dim_size)}"
    counts = np.bincount(index, minlength=B).astype(np.int64)
    offsets = np.concatenate([[0], np.cumsum(counts)[:-1]])
    lens = np.minimum(counts, MAXLEN)

    order, len_c, n_c, N_t, blocks, snap, masks = _plan(lens)

    # --- weights (torch gate order i,f,g,o) ---
    b = (b_ih + b_hh).reshape(4, H)
    Wi, Wf, Wg, Wo = W_ih.reshape(4, H, F)
    Ui, Uf, Ug, Uo = W_hh.reshape(4, H, H)
    # rhs rows 0:64 carry h; rows 64:128 carry x.
    def stack2(gA, gB):
        return np.concatenate(
            [np.concatenate([gA[0].T, gB[0].T], 1),
             np.concatenate([gA[1].T, gB[1].T], 1)], 0).astype(np.float16)
    # bank A = (f, 2*g)  [s = sig(2g)];  bank B = (o, i)
    wa = stack2((Uf, Wf), (2.0 * Ug, 2.0 * Wg))
    wb = stack2((Uo, Wo), (Ui, Wi))
    ba = np.concatenate([b[1], 2.0 * b[2]]).reshape(128, 1).astype(np.float32)
    bb = np.concatenate([b[3], b[0]]).reshape(128, 1).astype(np.float32)

    # --- per-core dense input, feature-major [64, rows] ---
    x16 = x.astype(np.float16)
    in_maps = []
    for cN in range(NCORES):
        gids = order[np.arange(G) * NCORES + cN]
        lens_cj = len_c[cN]
        offs_cj = offsets[gids]
        parts = []
        for (t0, nsteps, Wb, row0) in blocks:
            tsl = np.arange(t0, t0 + nsteps)
            node = offs_cj[:Wb, None] + tsl[None, :]             # [Wb, nsteps]
            valid = tsl[None, :] < lens_cj[:Wb, None]
            node = np.clip(node, 0, x.shape[0] - 1)
            blk = np.where(valid[:, :, None], x16[node], np.float16(0))
            parts.append(blk.transpose(1, 0, 2).reshape(nsteps * Wb, 64))
        xd = np.ascontiguousarray(np.concatenate(parts, axis=0).T)
        msk = np.ascontiguousarray(
            np.broadcast_to(masks[cN][None, :], (64, masks.shape[1])))
        in_maps.append({"xd": xd, "msk": msk, "wa": wa, "wb": wb,
                        "ba": ba, "bb": bb})

    import hashlib
    key = hashlib.sha1(
        (repr((N_t, blocks, snap))).encode()
        + W_ih.tobytes() + W_hh.tobytes() + b_ih.tobytes() + b_hh.tobytes()
    ).hexdigest()
    if key not in _CACHE:
        _CACHE[key] = _build_and_compile(
            (N_t, blocks, snap, masks.shape[1]), None)
    nc = _CACHE[key]

    res = run_bass_kernel_spmd(nc, in_maps, core_ids=list(range(NCORES)),
                               trace=_trace)
    LAST_RUN["res"] = res

    out = np.zeros((B, H), np.float32)
    for cN in range(NCORES):
        hT = res.results[cN]["outh"].astype(np.float32)  # [64, G]
        gids = order[np.arange(G) * NCORES + cN]
        out[gids] = hT.T
    return out


# revision 64
# speedup vs baseline: 1.0054x; 1.0054x over previous
"""Trainium2 Bass kernel: masked-LSTM readout over to_dense_batch'd graphs.

v5 strategy (8 NeuronCores, SPMD single program):
 - Host: per-graph lengths from sorted `index`; graphs globally sorted by
   length (desc) and dealt round-robin to 8 cores, so all cores share one
   step schedule N_t. Host densifies x into a block-major padded tensor per
   core (fp16), feature-major [64, rows].
 - Device per step: rhs = [h_{t-1} ; x_t] stacked on 128 partitions (h
   written into the x tile's top half by the previous step), so each
   gate-pair needs ONE matmul with contract 128. All four gates use one
   sigmoid form (g via s=sig(2g), per-partition ACT scale (1;2)); cell
   update in fp16: TT ops in DVE 2x mode, tanh(c) on ACT, f*c on GpSimd.
 - Two independent column pieces run PHASE-OFFSET by one step (piece0 a
   full step ahead in every engine queue) so the serial LSTM chain of one
   piece hides behind the other piece's engine work.
 - Final h snapshot via predicated copy at each graph's last valid step.
"""

import numpy as np

MAXLEN = 100
B = 8192
NCORES = 8
G = B // NCORES          # graph columns per core
H = 64
F = 64
TW = 24                  # steps per time block
CHUNK = 512              # psum bank width (f32 cols)

_CACHE = {}


def _split(rng, cuts):
    """Split [lo,hi) at the given cut points into ordered segments."""
    lo, hi = rng
    pts = sorted({lo, hi, *[c for c in cuts if lo < c < hi]})
    return list(zip(pts[:-1], pts[1:]))


def _build_and_compile(schedule, weights):
    import concourse.bacc as bacc
    import concourse.mybir as mybir
    from concourse import tile

    N_t, blocks, snap, MW = schedule
    fp16 = mybir.dt.float16
    f32 = mybir.dt.float32
    T_end = len(N_t)
    ROWS_TOT = sum(Wb * nst for (_, nst, Wb, _) in blocks)
    XT_W = max(Wb * nst for (_, nst, Wb, _) in blocks)

    nc = bacc.Bacc("TRN2", target_bir_lowering=False)
    xd_d = nc.dram_tensor("xd", [64, ROWS_TOT], fp16, kind="ExternalInput")
    msk_d = nc.dram_tensor("msk", [64, max(MW, 1)], mybir.dt.uint8, kind="ExternalInput")
    out_d = nc.dram_tensor("outh", [64, G], fp16, kind="ExternalOutput")
    # gate pairing: bank A = (f, g) [g-weights pre-doubled], bank B = (o, i)
    wa_d = nc.dram_tensor("wa", [128, 128], fp16, kind="ExternalInput")
    wb_d = nc.dram_tensor("wb", [128, 128], fp16, kind="ExternalInput")
    ba_d = nc.dram_tensor("ba", [128, 1], f32, kind="ExternalInput")
    bb_d = nc.dram_tensor("bb", [128, 1], f32, kind="ExternalInput")

    Sig = mybir.ActivationFunctionType.Sigmoid
    Tanh = mybir.ActivationFunctionType.Tanh
    Mult = mybir.AluOpType.mult
    Add = mybir.AluOpType.add

    blk_of = {}
    for bi, (t0, nst, Wb, row0) in enumerate(blocks):
        for ts in range(nst):
            blk_of[t0 + ts] = (bi, ts)

    # per-step metadata
    meta = []
    for t in range(T_end):
        n = N_t[t]
        if n == 0:
            break
        bi, ts = blk_of[t]
        Wb = blocks[bi][2]
        if n > 32:
            # balanced split at every width: both chains get ~n/2 columns so
            # they hide each other even in the mid-taper (n slightly > 512)
            m = min(CHUNK, (n // 2 + 15) & ~15)
            pieces = [(0, m), (m, n)]
        else:
            pieces = [(0, n)]
        if t + 1 < T_end and N_t[t + 1] > 0:
            nbi, nts = blk_of[t + 1]
            base_n = nts * blocks[nbi][2]
            wA = min(n, blocks[nbi][2])
        else:
            nbi, base_n, wA = None, 0, 0
        meta.append(dict(t=t, n=n, bi=bi, base=ts * Wb, pieces=pieces,
                         nbi=nbi, base_n=base_n, wA=wA))
    T = len(meta)

    with tile.TileContext(nc) as tc:
        with tc.tile_pool(name="state", bufs=1) as sp, \
             tc.tile_pool(name="xblk", bufs=2) as xp, \
             tc.tile_pool(name="psum", bufs=2, space="PSUM") as pp, \
             tc.tile_pool(name="gates", bufs=2) as gp:
            wa = sp.tile([128, 128], fp16)
            nc.sync.dma_start(out=wa, in_=wa_d.ap())
            wb = sp.tile([128, 128], fp16)
            nc.sync.dma_start(out=wb, in_=wb_d.ap())
            ba = sp.tile([128, 1], f32)
            nc.sync.dma_start(out=ba, in_=ba_d.ap())
            bb = sp.tile([128, 1], f32)
            nc.sync.dma_start(out=bb, in_=bb_d.ap())
            mskt = sp.tile([64, max(MW, 1)], mybir.dt.uint8)
            nc.sync.dma_start(out=mskt, in_=msk_d.ap())

            c = sp.tile([64, 1024], fp16, name="c")
            t1 = sp.tile([64, 1024], fp16, name="t1")
            t2 = sp.tile([64, 1024], fp16, name="t2")
            gt = sp.tile([128, 1024], fp16, name="gt")   # g~ on partitions 64:128
            tcc = sp.tile([64, 1024], fp16, name="tcc")
            hs = sp.tile([64, 1024], fp16, name="hs")
            outh = sp.tile([64, 1024], fp16, name="outh")
            nc.vector.memset(c[:, :], 0.0)
            nc.vector.memset(outh[:, :], 0.0)
            nc.vector.memset(hs[:, :], 0.0)

            xts = {}
            sgs = {}   # live sigma-output tiles keyed (t, pi)
            pss = {}
            pending_cps = []

            def make_xt(bi2):
                if bi2 in xts or bi2 >= len(blocks):
                    return
                _, nst2, Wb2, row02 = blocks[bi2]
                rows2 = Wb2 * nst2
                xt2 = xp.tile([128, XT_W], fp16, tag="xt", name=f"xt{bi2}")
                xts[bi2] = xt2
                # x occupies partitions 64:128; h occupies 0:64
                nc.sync.dma_start(out=xt2[64:128, 0:rows2],
                                  in_=xd_d.ap()[:, row02:row02 + rows2])
                if bi2 == 0:
                    nc.vector.memset(xt2[0:64, 0:Wb2], 0.0)

            def emit_A(k, pi):
                """MM + sigma + front of DVE chain for (step k, piece pi)."""
                st = meta[k]
                if pi >= len(st["pieces"]):
                    return
                lo, hi = st["pieces"][pi]
                w = hi - lo
                bi, base = st["bi"], st["base"]
                if pi == 0:
                    make_xt(bi)
                    make_xt(bi + 1)
                xt = xts[bi]
                # separate psum tiles per gate bank so sigA only waits on MM_A
                psa = pp.tile([128, 512], f32, tag=f"pa{pi}", name=f"pa{pi}_{k}")
                psb = pp.tile([128, 512], f32, tag=f"pb{pi}", name=f"pb{pi}_{k}")
                pss[(k, pi)] = (psa, psb)
                # bank A = (f, g): unlocks TS->TT2 and TT1 immediately
                nc.tensor.matmul(out=psa[:, 0:w], lhsT=wa[:, :],
                                 rhs=xt[:, base + lo:base + hi],
                                 start=True, stop=True)
                nc.tensor.matmul(out=psb[:, 0:w], lhsT=wb[:, :],
                                 rhs=xt[:, base + lo:base + hi],
                                 start=True, stop=True)
                sg = gp.tile([128, 1024], fp16, tag=f"sg{pi}", name=f"sg{pi}_{k}")
                sgs[(k, pi)] = sg
                nc.scalar.activation(out=sg[:, 0:w], in_=psa[:, 0:w],
                                     func=Sig, bias=ba[:, :])
                nc.scalar.activation(out=sg[:, CHUNK:CHUNK + w],
                                     in_=psb[:, 0:w],
                                     func=Sig, bias=bb[:, :])
                # g~ = 2*s - 1 (s = sig(2g) at A rows 64:128)
                nc.vector.tensor_scalar(
                    out=gt[64:128, lo:hi], in0=sg[64:128, 0:w],
                    scalar1=2.0, scalar2=-1.0, op0=Mult, op1=Add)
                # t1 = sig(f) * c
                nc.vector.tensor_tensor(
                    out=t1[:, lo:hi], in0=sg[0:64, 0:w],
                    in1=c[:, lo:hi], op=Mult)
                # t2 = sig(i) * g~   (i = B rows 64:128)
                nc.vector.tensor_tensor(
                    out=t2[:, lo:hi], in0=sg[64:128, CHUNK:CHUNK + w],
                    in1=gt[64:128, lo:hi], op=Mult)
                nc.vector.tensor_tensor(
                    out=c[:, lo:hi], in0=t1[:, lo:hi],
                    in1=t2[:, lo:hi], op=Add)

            def emit_B(k, pi):
                """tanh + h-update + snapshot for (step k, piece pi)."""
                st = meta[k]
                if pi >= len(st["pieces"]):
                    return
                lo, hi = st["pieces"][pi]
                wA, base_n, nbi = st["wA"], st["base_n"], st["nbi"]
                xt_n = xts[nbi] if nbi is not None else None
                sg = sgs[(k, pi)]
                nc.scalar.activation(out=tcc[:, lo:hi], in_=c[:, lo:hi],
                                     func=Tanh)
                # h = sig(o) * tanh(c)   (o = B rows 0:64)
                for (a, b_) in _split((lo, hi), [wA]):
                    if b_ <= wA:
                        dst = xt_n[0:64, base_n + a:base_n + b_]
                    else:
                        dst = hs[:, a:b_]
                    nc.vector.tensor_tensor(
                        out=dst, in0=sg[0:64, CHUNK + a - lo:CHUNK + b_ - lo],
                        in1=tcc[:, a:b_], op=Mult)
                # snapshot graphs whose sequence ends at step k: batched and
                # flushed at block end to keep CPs out of the hot DVE window
                for (slo, shi, moff) in snap[st["t"]]:
                    ilo, ihi = max(slo, lo), min(shi, hi)
                    if ihi <= ilo:
                        continue
                    for (a, b_) in _split((ilo, ihi), [wA]):
                        if b_ <= wA:
                            src = xt_n[0:64, base_n + a:base_n + b_]
                        else:
                            src = hs[:, a:b_]
                        pending_cps.append((a, b_, moff + a - slo, src))

            def flush_cps():
                for (a, b_, mo, src) in pending_cps:
                    nc.vector.copy_predicated(
                        out=outh[:, a:b_], mask=mskt[:, mo:mo + b_ - a],
                        data=src)
                pending_cps.clear()

            # phase-offset emission: piece0 runs one step ahead of piece1.
            # Each piece's B-part (tanh/h-update) queues directly behind its
            # own A-part chain so TT4 is never stuck behind the other piece.
            emit_A(0, 0)
            for k in range(T):
                emit_B(k, 0)
                emit_A(k, 1)
                emit_B(k, 1)
                if k + 1 < T:
                    emit_A(k + 1, 0)
                if k + 1 >= T or meta[k + 1]["bi"] != meta[k]["bi"]:
                    flush_cps()

            nc.sync.dma_start(out=out_d.ap()[:, 0:G], in_=outh[:, 0:G])
    nc.compile()
    return nc


def _plan(lens):
    """Global schedule from capped lengths [B]."""
    order = np.argsort(-lens, kind="stable")
    lens_sorted = lens[order]
    T_end = int(lens_sorted.max())
    len_c = lens_sorted.reshape(G, NCORES).T            # [NCORES, G]
    t_ax = np.arange(T_end + 1)
    n_c = (len_c[:, :, None] > t_ax[None, None, :]).sum(axis=1)
    N_t = n_c.max(axis=0)                               # [T_end+1], N_t[T_end]==0
    blocks = []
    row0 = 0
    t0 = 0
    # graduated early blocks: each block's DMA must land before its first
    # step; compute ramps faster than the full-size prefetch pipeline
    grad = [4, 8, 16]
    while t0 < T_end:
        nsteps = min(grad[len(blocks)] if len(blocks) < len(grad) else TW,
                     T_end - t0)
        Wb = max(16, int(np.ceil(N_t[t0] / 16) * 16))
        blocks.append((t0, nsteps, Wb, row0))
        row0 += Wb * nsteps
        t0 += nsteps
    snap = []
    moff = 0
    mask_cols = []
    for t in range(T_end):
        nt1 = n_c[:, t + 1]
        lo = int(nt1.min())
        hi = int(n_c[:, t].max())
        pieces = []
        if hi > lo:
            m = np.zeros((NCORES, hi - lo), np.uint8)
            for cc in range(NCORES):
                a, b_ = int(nt1[cc]), int(n_c[cc, t])
                m[cc, max(a - lo, 0):max(b_ - lo, 0)] = 1
            mask_cols.append(m)
            pieces.append((lo, hi, moff))
            moff += hi - lo
        snap.append(pieces)
    masks = (np.concatenate(mask_cols, axis=1) if mask_cols
             else np.zeros((NCORES, 1), np.uint8))
    return order, len_c, n_c, [int(x) for x in N_t[:T_end]], blocks, snap, masks


LAST_RUN = {}


def _install_ntff_shim():
    import sys, types
    if "antenv.axon_hooks" in sys.modules:
        return
    try:
        from trn_agent_boot.trn_boot import _ntff_profile_via_ctypes
        hook = _ntff_profile_via_ctypes("/opt/axon/libaxon_pjrt.so")
    except Exception:
        hook = None
    m = types.ModuleType("antenv.axon_hooks")
    m._hook = hook
    m.get_axon_ntff_profile_hook = lambda: m._hook
    m.set_axon_ntff_profile_hook = lambda h: setattr(m, "_hook", h)
    sys.modules["antenv.axon_hooks"] = m


def kernel(x, W_ih, W_hh, b_ih, b_hh, index, dim_size, _trace=False):
    from concourse.bass_utils import run_bass_kernel_spmd
    if _trace:
        import concourse.bass_utils as _bu
        _install_ntff_shim()
        _bu.upload_artifacts = lambda d: d

    x = np.asarray(x)
    index = np.asarray(index).astype(np.int64)
    W_ih = np.asarray(W_ih, dtype=np.float32)
    W_hh = np.asarray(W_hh, dtype=np.float32)
    b_ih = np.asarray(b_ih, dtype=np.float32)
    b_hh = np.asarray(b_hh, dtype=np.float32)

    assert int(dim_size) == B, f"kernel hardcodes B={B}, got dim_size={int(dim_size)}"
    counts = np.bincount(index, minlength=B).astype(np.int64)
    offsets = np.concatenate([[0], np.cumsum(counts)[:-1]])
    lens = np.minimum(counts, MAXLEN)

    order, len_c, n_c, N_t, blocks, snap, masks = _plan(lens)

    # --- weights (torch gate order i,f,g,o) ---
    b = (b_ih + b_hh).reshape(4, H)
    Wi, Wf, Wg, Wo = W_ih.reshape(4, H, F)
    Ui, Uf, Ug, Uo = W_hh.reshape(4, H, H)
    # rhs rows 0:64 carry h; rows 64:128 carry x.
    def stack2(gA, gB):
        return np.concatenate(
            [np.concatenate([gA[0].T, gB[0].T], 1),
             np.concatenate([gA[1].T, gB[1].T], 1)], 0).astype(np.float16)
    # bank A = (f, 2*g)  [s = sig(2g)];  bank B = (o, i)
    wa = stack2((Uf, Wf), (2.0 * Ug, 2.0 * Wg))
    wb = stack2((Uo, Wo), (Ui, Wi))
    ba = np.concatenate([b[1], 2.0 * b[2]]).reshape(128, 1).astype(np.float32)
    bb = np.concatenate([b[3], b[0]]).reshape(128, 1).astype(np.float32)

    # --- per-core dense input, feature-major [64, rows] ---
    x16 = x.astype(np.float16)
    in_maps = []
    for cN in range(NCORES):
        gids = order[np.arange(G) * NCORES + cN]
        lens_cj = len_c[cN]
        offs_cj = offsets[gids]
        parts = []
        for (t0, nsteps, Wb, row0) in blocks:
            tsl = np.arange(t0, t0 + nsteps)
            node = offs_cj[:Wb, None] + tsl[None, :]             # [Wb, nsteps]
            valid = tsl[None, :] < lens_cj[:Wb, None]
            node = np.clip(node, 0, x.shape[0] - 1)
            blk = np.where(valid[:, :, None], x16[node], np.float16(0))
            parts.append(blk.transpose(1, 0, 2).reshape(nsteps * Wb, 64))
        xd = np.ascontiguousarray(np.concatenate(parts, axis=0).T)
        msk = np.ascontiguousarray(
            np.broadcast_to(masks[cN][None, :], (64, masks.shape[1])))
        in_maps.append({"xd": xd, "msk": msk, "wa": wa, "wb": wb,
                        "ba": ba, "bb": bb})

    import hashlib
    key = hashlib.sha1(
        (repr((N_t, blocks, snap))).encode()
        + W_ih.tobytes() + W_hh.tobytes() + b_ih.tobytes() + b_hh.tobytes()
    ).hexdigest()
    if key not in _CACHE:
        _CACHE[key] = _build_and_compile(
            (N_t, blocks, snap, masks.shape[1]), None)
    nc = _CACHE[key]

    res = run_bass_kernel_spmd(nc, in_maps, core_ids=list(range(NCORES)),
                               trace=_trace)
    LAST_RUN["res"] = res

    out = np.zeros((B, H), np.float32)
    for cN in range(NCORES):
        hT = res.results[cN]["outh"].astype(np.float32)  # [64, G]
        gids = order[np.arange(G) * NCORES + cN]
        out[gids] = hT.T
    return out


# revision 65
# speedup vs baseline: 1.0065x; 1.0011x over previous
"""Trainium2 Bass kernel: masked-LSTM readout over to_dense_batch'd graphs.

v5 strategy (8 NeuronCores, SPMD single program):
 - Host: per-graph lengths from sorted `index`; graphs globally sorted by
   length (desc) and dealt round-robin to 8 cores, so all cores share one
   step schedule N_t. Host densifies x into a block-major padded tensor per
   core (fp16), feature-major [64, rows].
 - Device per step: rhs = [h_{t-1} ; x_t] stacked on 128 partitions (h
   written into the x tile's top half by the previous step), so each
   gate-pair needs ONE matmul with contract 128. All four gates use one
   sigmoid form (g via s=sig(2g), per-partition ACT scale (1;2)); cell
   update in fp16: TT ops in DVE 2x mode, tanh(c) on ACT, f*c on GpSimd.
 - Two independent column pieces run PHASE-OFFSET by one step (piece0 a
   full step ahead in every engine queue) so the serial LSTM chain of one
   piece hides behind the other piece's engine work.
 - Final h snapshot via predicated copy at each graph's last valid step.
"""

import numpy as np

MAXLEN = 100
B = 8192
NCORES = 8
G = B // NCORES          # graph columns per core
H = 64
F = 64
TW = 24                  # steps per time block
CHUNK = 512              # psum bank width (f32 cols)

_CACHE = {}


def _split(rng, cuts):
    """Split [lo,hi) at the given cut points into ordered segments."""
    lo, hi = rng
    pts = sorted({lo, hi, *[c for c in cuts if lo < c < hi]})
    return list(zip(pts[:-1], pts[1:]))


def _build_and_compile(schedule, weights):
    import concourse.bacc as bacc
    import concourse.mybir as mybir
    from concourse import tile

    N_t, blocks, snap, MW = schedule
    fp16 = mybir.dt.float16
    f32 = mybir.dt.float32
    T_end = len(N_t)
    ROWS_TOT = sum(Wb * nst for (_, nst, Wb, _) in blocks)
    XT_W = max(Wb * nst for (_, nst, Wb, _) in blocks)

    nc = bacc.Bacc("TRN2", target_bir_lowering=False)
    xd_d = nc.dram_tensor("xd", [64, ROWS_TOT], fp16, kind="ExternalInput")
    msk_d = nc.dram_tensor("msk", [64, max(MW, 1)], mybir.dt.uint8, kind="ExternalInput")
    out_d = nc.dram_tensor("outh", [64, G], fp16, kind="ExternalOutput")
    # gate pairing: bank A = (f, g) [g-weights pre-doubled], bank B = (o, i)
    wa_d = nc.dram_tensor("wa", [128, 128], fp16, kind="ExternalInput")
    wb_d = nc.dram_tensor("wb", [128, 128], fp16, kind="ExternalInput")
    ba_d = nc.dram_tensor("ba", [128, 1], f32, kind="ExternalInput")
    bb_d = nc.dram_tensor("bb", [128, 1], f32, kind="ExternalInput")

    Sig = mybir.ActivationFunctionType.Sigmoid
    Tanh = mybir.ActivationFunctionType.Tanh
    Mult = mybir.AluOpType.mult
    Add = mybir.AluOpType.add

    blk_of = {}
    for bi, (t0, nst, Wb, row0) in enumerate(blocks):
        for ts in range(nst):
            blk_of[t0 + ts] = (bi, ts)

    # per-step metadata
    meta = []
    for t in range(T_end):
        n = N_t[t]
        if n == 0:
            break
        bi, ts = blk_of[t]
        Wb = blocks[bi][2]
        if n > 32:
            # balanced split at every width: both chains get ~n/2 columns so
            # they hide each other even in the mid-taper (n slightly > 512)
            m = min(CHUNK, (n // 2 + 15) & ~15)
            pieces = [(0, m), (m, n)]
        else:
            pieces = [(0, n)]
        if t + 1 < T_end and N_t[t + 1] > 0:
            nbi, nts = blk_of[t + 1]
            base_n = nts * blocks[nbi][2]
            wA = min(n, blocks[nbi][2])
        else:
            nbi, base_n, wA = None, 0, 0
        meta.append(dict(t=t, n=n, bi=bi, base=ts * Wb, pieces=pieces,
                         nbi=nbi, base_n=base_n, wA=wA))
    T = len(meta)

    with tile.TileContext(nc) as tc:
        with tc.tile_pool(name="state", bufs=1) as sp, \
             tc.tile_pool(name="xblk", bufs=2) as xp, \
             tc.tile_pool(name="psum", bufs=2, space="PSUM") as pp, \
             tc.tile_pool(name="gates", bufs=2) as gp:
            wa = sp.tile([128, 128], fp16)
            nc.sync.dma_start(out=wa, in_=wa_d.ap())
            wb = sp.tile([128, 128], fp16)
            nc.sync.dma_start(out=wb, in_=wb_d.ap())
            ba = sp.tile([128, 1], f32)
            nc.sync.dma_start(out=ba, in_=ba_d.ap())
            bb = sp.tile([128, 1], f32)
            nc.sync.dma_start(out=bb, in_=bb_d.ap())
            mskt = sp.tile([64, max(MW, 1)], mybir.dt.uint8)
            nc.sync.dma_start(out=mskt, in_=msk_d.ap())

            c = sp.tile([64, 1024], fp16, name="c")
            t1 = sp.tile([64, 1024], fp16, name="t1")
            t2 = sp.tile([64, 1024], fp16, name="t2")
            gt = sp.tile([128, 1024], fp16, name="gt")   # g~ on partitions 64:128
            tcc = sp.tile([64, 1024], fp16, name="tcc")
            hs = sp.tile([64, 1024], fp16, name="hs")
            outh = sp.tile([64, 1024], fp16, name="outh")
            nc.vector.memset(c[:, :], 0.0)
            nc.vector.memset(outh[:, :], 0.0)
            nc.vector.memset(hs[:, :], 0.0)

            xts = {}
            sgs = {}   # live sigma-output tiles keyed (t, pi)
            pss = {}
            pending_cps = []

            def make_xt(bi2):
                if bi2 in xts or bi2 >= len(blocks):
                    return
                _, nst2, Wb2, row02 = blocks[bi2]
                rows2 = Wb2 * nst2
                xt2 = xp.tile([128, XT_W], fp16, tag="xt", name=f"xt{bi2}")
                xts[bi2] = xt2
                # x occupies partitions 64:128; h occupies 0:64
                nc.sync.dma_start(out=xt2[64:128, 0:rows2],
                                  in_=xd_d.ap()[:, row02:row02 + rows2])
                if bi2 == 0:
                    nc.vector.memset(xt2[0:64, 0:Wb2], 0.0)

            def emit_A(k, pi):
                """MM + sigma + front of DVE chain for (step k, piece pi)."""
                st = meta[k]
                if pi >= len(st["pieces"]):
                    return
                lo, hi = st["pieces"][pi]
                w = hi - lo
                bi, base = st["bi"], st["base"]
                if pi == 0:
                    make_xt(bi)
                    make_xt(bi + 1)
                xt = xts[bi]
                # separate psum tiles per gate bank so sigA only waits on MM_A
                psa = pp.tile([128, 512], f32, tag=f"pa{pi}", name=f"pa{pi}_{k}")
                psb = pp.tile([128, 512], f32, tag=f"pb{pi}", name=f"pb{pi}_{k}")
                pss[(k, pi)] = (psa, psb)
                # bank A = (f, g): unlocks TS->TT2 and TT1 immediately
                nc.tensor.matmul(out=psa[:, 0:w], lhsT=wa[:, :],
                                 rhs=xt[:, base + lo:base + hi],
                                 start=True, stop=True)
                nc.tensor.matmul(out=psb[:, 0:w], lhsT=wb[:, :],
                                 rhs=xt[:, base + lo:base + hi],
                                 start=True, stop=True)
                sg = gp.tile([128, 1024], fp16, tag=f"sg{pi}", name=f"sg{pi}_{k}")
                sgs[(k, pi)] = sg
                nc.scalar.activation(out=sg[:, 0:w], in_=psa[:, 0:w],
                                     func=Sig, bias=ba[:, :])
                nc.scalar.activation(out=sg[:, CHUNK:CHUNK + w],
                                     in_=psb[:, 0:w],
                                     func=Sig, bias=bb[:, :])
                # g~ = 2*s - 1 (s = sig(2g) at A rows 64:128)
                nc.vector.tensor_scalar(
                    out=gt[64:128, lo:hi], in0=sg[64:128, 0:w],
                    scalar1=2.0, scalar2=-1.0, op0=Mult, op1=Add)
                # t1 = sig(f) * c
                nc.vector.tensor_tensor(
                    out=t1[:, lo:hi], in0=sg[0:64, 0:w],
                    in1=c[:, lo:hi], op=Mult)
                # t2 = sig(i) * g~   (i = B rows 64:128)
                nc.vector.tensor_tensor(
                    out=t2[:, lo:hi], in0=sg[64:128, CHUNK:CHUNK + w],
                    in1=gt[64:128, lo:hi], op=Mult)
                nc.vector.tensor_tensor(
                    out=c[:, lo:hi], in0=t1[:, lo:hi],
                    in1=t2[:, lo:hi], op=Add)

            def emit_B(k, pi):
                """tanh + h-update + snapshot for (step k, piece pi)."""
                st = meta[k]
                if pi >= len(st["pieces"]):
                    return
                lo, hi = st["pieces"][pi]
                wA, base_n, nbi = st["wA"], st["base_n"], st["nbi"]
                xt_n = xts[nbi] if nbi is not None else None
                sg = sgs[(k, pi)]
                nc.scalar.activation(out=tcc[:, lo:hi], in_=c[:, lo:hi],
                                     func=Tanh)
                # h = sig(o) * tanh(c)   (o = B rows 0:64)
                for (a, b_) in _split((lo, hi), [wA]):
                    if b_ <= wA:
                        dst = xt_n[0:64, base_n + a:base_n + b_]
                    else:
                        dst = hs[:, a:b_]
                    nc.vector.tensor_tensor(
                        out=dst, in0=sg[0:64, CHUNK + a - lo:CHUNK + b_ - lo],
                        in1=tcc[:, a:b_], op=Mult)
                # snapshot graphs whose sequence ends at step k: batched and
                # flushed at block end to keep CPs out of the hot DVE window
                for (slo, shi, moff) in snap[st["t"]]:
                    ilo, ihi = max(slo, lo), min(shi, hi)
                    if ihi <= ilo:
                        continue
                    for (a, b_) in _split((ilo, ihi), [wA]):
                        if b_ <= wA:
                            src = xt_n[0:64, base_n + a:base_n + b_]
                        else:
                            src = hs[:, a:b_]
                        pending_cps.append((a, b_, moff + a - slo, src))

            def flush_cps():
                for (a, b_, mo, src) in pending_cps:
                    nc.vector.copy_predicated(
                        out=outh[:, a:b_], mask=mskt[:, mo:mo + b_ - a],
                        data=src)
                pending_cps.clear()

            # phase-offset emission: piece0 runs one step ahead of piece1.
            # Each piece's B-part (tanh/h-update) queues directly behind its
            # own A-part chain so TT4 is never stuck behind the other piece.
            emit_A(0, 0)
            for k in range(T):
                emit_B(k, 0)
                emit_A(k, 1)
                emit_B(k, 1)
                if k + 1 < T:
                    emit_A(k + 1, 0)
                if k + 1 >= T or meta[k + 1]["bi"] != meta[k]["bi"]:
                    flush_cps()

            nc.sync.dma_start(out=out_d.ap()[:, 0:G], in_=outh[:, 0:G])
    nc.compile()
    return nc


def _plan(lens):
    """Global schedule from capped lengths [B]."""
    order = np.argsort(-lens, kind="stable")
    lens_sorted = lens[order]
    T_end = int(lens_sorted.max())
    len_c = lens_sorted.reshape(G, NCORES).T            # [NCORES, G]
    t_ax = np.arange(T_end + 1)
    n_c = (len_c[:, :, None] > t_ax[None, None, :]).sum(axis=1)
    N_t = n_c.max(axis=0)                               # [T_end+1], N_t[T_end]==0
    blocks = []
    row0 = 0
    t0 = 0
    while t0 < T_end:
        # small first block so compute starts as soon as its DMA lands
        nsteps = min(4 if t0 == 0 else TW, T_end - t0)
        Wb = max(16, int(np.ceil(N_t[t0] / 16) * 16))
        blocks.append((t0, nsteps, Wb, row0))
        row0 += Wb * nsteps
        t0 += nsteps
    snap = []
    moff = 0
    mask_cols = []
    for t in range(T_end):
        nt1 = n_c[:, t + 1]
        lo = int(nt1.min())
        hi = int(n_c[:, t].max())
        pieces = []
        if hi > lo:
            m = np.zeros((NCORES, hi - lo), np.uint8)
            for cc in range(NCORES):
                a, b_ = int(nt1[cc]), int(n_c[cc, t])
                m[cc, max(a - lo, 0):max(b_ - lo, 0)] = 1
            mask_cols.append(m)
            pieces.append((lo, hi, moff))
            moff += hi - lo
        snap.append(pieces)
    masks = (np.concatenate(mask_cols, axis=1) if mask_cols
             else np.zeros((NCORES, 1), np.uint8))
    return order, len_c, n_c, [int(x) for x in N_t[:T_end]], blocks, snap, masks


LAST_RUN = {}


def _install_ntff_shim():
    import sys, types
    if "antenv.axon_hooks" in sys.modules:
        return
    try:
        from trn_agent_boot.trn_boot import _ntff_profile_via_ctypes
        hook = _ntff_profile_via_ctypes("/opt/axon/libaxon_pjrt.so")
    except Exception:
        hook = None
    m = types.ModuleType("antenv.axon_hooks")
    m._hook = hook
    m.get_axon_ntff_profile_hook = lambda: m._hook
    m.set_axon_ntff_profile_hook = lambda h: setattr(m, "_hook", h)
    sys.modules["antenv.axon_hooks"] = m


def kernel(x, W_ih, W_hh, b_ih, b_hh, index, dim_size, _trace=False):
    from concourse.bass_utils import run_bass_kernel_spmd
    if _trace:
        import concourse.bass_utils as _bu
        _install_ntff_shim()
        _bu.upload_artifacts = lambda d: d

    x = np.asarray(x)
    index = np.asarray(index).astype(np.int64)
    W_ih = np.asarray(W_ih, dtype=np.float32)
    W_hh = np.asarray(W_hh, dtype=np.float32)
    b_ih = np.asarray(b_ih, dtype=np.float32)
    b_hh = np.asarray(b_hh, dtype=np.float32)

    assert int(dim_size) == B, f"kernel hardcodes B={B}, got dim_size={int(dim_size)}"
    counts = np.bincount(index, minlength=B).astype(np.int64)
    offsets = np.concatenate([[0], np.cumsum(counts)[:-1]])
    lens = np.minimum(counts, MAXLEN)

    order, len_c, n_c, N_t, blocks, snap, masks = _plan(lens)

    # --- weights (torch gate order i,f,g,o) ---
    b = (b_ih + b_hh).reshape(4, H)
    Wi, Wf, Wg, Wo = W_ih.reshape(4, H, F)
    Ui, Uf, Ug, Uo = W_hh.reshape(4, H, H)
    # rhs rows 0:64 carry h; rows 64:128 carry x.
    def stack2(gA, gB):
        return np.concatenate(
            [np.concatenate([gA[0].T, gB[0].T], 1),
             np.concatenate([gA[1].T, gB[1].T], 1)], 0).astype(np.float16)
    # bank A = (f, 2*g)  [s = sig(2g)];  bank B = (o, i)
    wa = stack2((Uf, Wf), (2.0 * Ug, 2.0 * Wg))
    wb = stack2((Uo, Wo), (Ui, Wi))
    ba = np.concatenate([b[1], 2.0 * b[2]]).reshape(128, 1).astype(np.float32)
    bb = np.concatenate([b[3], b[0]]).reshape(128, 1).astype(np.float32)

    # --- per-core dense input, feature-major [64, rows] ---
    x16 = x.astype(np.float16)
    in_maps = []
    for cN in range(NCORES):
        gids = order[np.arange(G) * NCORES + cN]
        lens_cj = len_c[cN]
        offs_cj = offsets[gids]
        parts = []
        for (t0, nsteps, Wb, row0) in blocks:
            tsl = np.arange(t0, t0 + nsteps)
            node = offs_cj[:Wb, None] + tsl[None, :]             # [Wb, nsteps]
            valid = tsl[None, :] < lens_cj[:Wb, None]
            node = np.clip(node, 0, x.shape[0] - 1)
            blk = np.where(valid[:, :, None], x16[node], np.float16(0))
            parts.append(blk.transpose(1, 0, 2).reshape(nsteps * Wb, 64))
        xd = np.ascontiguousarray(np.concatenate(parts, axis=0).T)
        msk = np.ascontiguousarray(
            np.broadcast_to(masks[cN][None, :], (64, masks.shape[1])))
        in_maps.append({"xd": xd, "msk": msk, "wa": wa, "wb": wb,
                        "ba": ba, "bb": bb})

    import hashlib
    key = hashlib.sha1(
        (repr((N_t, blocks, snap))).encode()
        + W_ih.tobytes() + W_hh.tobytes() + b_ih.tobytes() + b_hh.tobytes()
    ).hexdigest()
    if key not in _CACHE:
        _CACHE[key] = _build_and_compile(
            (N_t, blocks, snap, masks.shape[1]), None)
    nc = _CACHE[key]

    res = run_bass_kernel_spmd(nc, in_maps, core_ids=list(range(NCORES)),
                               trace=_trace)
    LAST_RUN["res"] = res

    out = np.zeros((B, H), np.float32)
    for cN in range(NCORES):
        hT = res.results[cN]["outh"].astype(np.float32)  # [64, G]
        gids = order[np.arange(G) * NCORES + cN]
        out[gids] = hT.T
    return out
